# revision 7
# baseline (speedup 1.0000x reference)
"""Trainium2 Bass kernel for BasicRecurrentEntityEncoder (v2).

Data-parallel over paragraphs: 8 cores x 8 paragraphs. Columns layout
[d=128 partitions, n=160 free] with n = b_local*20 + k.

Phase A (pipelined under phase B): embedding gather via indirect DMA
(masked tokens -> appended zero row), position-weighted sentence sums via
fp16 block matmuls into PSUM -> enc16 [128, 512]; E_rep [128, 10240] fp16
(e broadcast over the 20 entity slots, per step); EK bias table
ek[s,n] = sum_d e_s[d,b]*keys[d,(b,k)] + mask_bias (fp32), flattened to a
[1, 64*160] partition-0 row via SBUF->SBUF DMA so each step's bias can be
injected into the gate PSUM with a 1x128-contraction row matmul.

Phase B: 64 serial steps, all matmuls fp16 (single-pass + fast weight
load; fp32 was 2-pass and dominated the baseline):
  gate   g = sigmoid(ones16^T @ (h16*E_s) + ek_row_s)   [ACT Sigmoid
         reads PSUM; masked sentences get -6e4 bias; the sigmoid spline
         returns exactly 0.0 below its leftmost bucket (HW-verified), so
         masked steps are exact no-ops]
  cand   HT = relu(U^T h16 + V^T keysT + W^T e_rep_s)   [ACT Relu]
  update y = h + g*HT
  norm   ss = halfones^T @ y^2 + 0.5e-12  (-> hs = 0.5*(ss+eps) in PSUM)
         rs = rsqrt via fp32 bit-trick seed + 2 Newton steps on DVE
         (classic 0x5f3759df with the magic pre-adjusted for the 0.5x
         input; STT ops keep each Newton step at 3 instructions)
  h' = y * rs
"""

import numpy as np

B, S, L, K, D, VOC = 64, 64, 20, 20, 128, 50000
NCORES = 8
BL = B // NCORES          # paragraphs per core = 8
N = BL * K                # recurrence columns = 160
TOK = BL * S * L          # gathered tokens per core = 10240
TPB = 120                 # tokens per block (6 whole sentences)
NBLK = (TOK + TPB - 1) // TPB   # 86 blocks (last padded with zero rows)
CHUNKS = [11, 11, 11, 11, 11, 11, 10, 10]   # gather/encode groups (blocks)
ZROW = VOC                # index of appended all-zero embedding row
GATE_BIAS = -6.0e4        # mask bias: sigmoid(-6e4) == exactly 0.0 on HW
M2 = 0x5f3759df - 0x00400000  # rsqrt magic, adjusted for hs = 0.5*x input
NR_ITERS = 1

# fp16 const tile layout (d_c16 [128, C16W])
C16_ONES = 0      # [:, 0:128] ones
C16_HALF = 128    # [:, 128:256] 0.5
C16_U = 256       # [:, 256:384]
C16_V = 384       # [:, 384:512]
C16_W = 512       # [:, 512:640]
C16_KEYS = 640    # [:, 640:800] keysT
C16_OMAP = 800    # [0:120, 800:806] block->sentence map
C16W = 806
# fp32 const tile layout (d_c32 [128, C32W])
C32_POSW = 0      # [0:120, 0:128] position weights per block row
C32_ROW = 128     # [0:1, 128:256] ones row (fp32 row-matmul lhsT)
C32_EPS = 256     # [0:1, 256:416] eps row: 0.5e-12 (rhs of row-matmul)
C32W = 416

_NC_CACHE = {}


def _build_nc():
    import concourse.bass as bass
    import concourse.tile as tile
    from concourse import mybir

    f32 = mybir.dt.float32
    f16 = mybir.dt.float16
    i32 = mybir.dt.int32
    AF = mybir.ActivationFunctionType
    OP = mybir.AluOpType

    nc = bass.Bass()

    d_emb = nc.declare_dram_parameter("emb", [VOC + 1, D], f32, isOutput=False)
    d_idx = nc.declare_dram_parameter("idx", [TPB, NBLK], i32, isOutput=False)
    d_mb = nc.declare_dram_parameter("mb", [8, (S // 8) * N], f32, isOutput=False)
    d_c16 = nc.declare_dram_parameter("c16", [128, C16W], f16, isOutput=False)
    d_c32 = nc.declare_dram_parameter("c32", [128, C32W], f32, isOutput=False)
    d_out = nc.declare_dram_parameter("out", [D, N], f32, isOutput=True)

    from contextlib import ExitStack
    with ExitStack() as ctx:
        tc = ctx.enter_context(tile.TileContext(nc))
        singles = ctx.enter_context(tc.tile_pool(name="singles", bufs=1))
        wpool = ctx.enter_context(tc.tile_pool(name="wtile", bufs=3))
        ekpool = ctx.enter_context(tc.tile_pool(name="ektile", bufs=2))
        step_sb = ctx.enter_context(tc.tile_pool(name="step_sb", bufs=2))
        hpool = ctx.enter_context(tc.tile_pool(name="hpool", bufs=2))
        p_enc = ctx.enter_context(tc.tile_pool(name="p_enc", bufs=1, space="PSUM"))
        p_ek = ctx.enter_context(tc.tile_pool(name="p_ek", bufs=1, space="PSUM"))
        p_q = ctx.enter_context(tc.tile_pool(name="p_q", bufs=2, space="PSUM"))
        p_ht = ctx.enter_context(tc.tile_pool(name="p_ht", bufs=2, space="PSUM"))
        p_ss = ctx.enter_context(tc.tile_pool(name="p_ss", bufs=2, space="PSUM"))

        # ---- constants ----
        idx_sb = singles.tile([TPB, NBLK], i32)
        nc.sync.dma_start(out=idx_sb[:, :], in_=d_idx[:, :])
        c16 = singles.tile([128, C16W], f16)
        nc.sync.dma_start(out=c16[:, :], in_=d_c16[:, :])
        c32 = singles.tile([128, C32W], f32)
        nc.sync.dma_start(out=c32[:, :], in_=d_c32[:, :])
        mb_sb = singles.tile([8, (S // 8) * N], f32)
        nc.sync.dma_start(out=mb_sb[:, :], in_=d_mb[:, :])

        ones16 = c16[:, C16_ONES:C16_ONES + 128]
        half16 = c16[:, C16_HALF:C16_HALF + 128]
        U16 = c16[:, C16_U:C16_U + 128]
        V16 = c16[:, C16_V:C16_V + 128]
        W16 = c16[:, C16_W:C16_W + 128]
        keysT16 = c16[:, C16_KEYS:C16_KEYS + N]
        omap16 = c16[0:TPB, C16_OMAP:C16_OMAP + 6]
        posw = c32[0:TPB, C32_POSW:C32_POSW + 128]
        row1 = c32[0:1, C32_ROW:C32_ROW + 128]
        epsrow = c32[0:1, C32_EPS:C32_EPS + N]

        # engine warmups (one tiny op per engine so phase B waits are short)
        warm = singles.tile([1, 8], f32)
        nc.vector.tensor_copy(out=warm[0:1, 0:1], in_=c32[0:1, 0:1])
        nc.scalar.copy(out=warm[0:1, 1:2], in_=c32[0:1, 0:1])
        nc.gpsimd.tensor_copy(out=warm[0:1, 2:3], in_=c32[0:1, 0:1])

        G_sb = singles.tile([TPB, NBLK * D], f32)      # gathered token rows
        enc16 = singles.tile([128, S * BL], f16)       # encoded sentences fp16
        E_rep = singles.tile([128, S * N], f16)        # e broadcast over k
        ekflat = singles.tile([1, S * N], f32)         # gate bias row table
        psum_enc = p_enc.tile([128, S * BL], f32)

        # ---- state init ----
        h16 = hpool.tile([D, N], f16, tag="h16")
        nc.vector.memset(h16[:, :], 0.0)

        # ---- Phase A: gather + position-weighted sentence sums ----
        # Every 120-token block holds 6 whole sentences; each sentence sum is
        # one start=True fp16 matmul into psum_enc.
        ek_groups_done = 0
        cols_done = 0

        def emit_ek_groups(upto_cols):
            # EK for sentence-groups of 8 once their enc16 columns exist.
            nonlocal ek_groups_done
            while (ek_groups_done + 1) * 8 * BL <= upto_cols:
                gidx = ek_groups_done
                s0 = gidx * 8
                psum_ek = p_ek.tile([8, N], f32, tag="ek")
                for b in range(BL):
                    lhs = enc16[:, s0 * BL + b:(s0 + 8) * BL:BL]
                    nc.tensor.matmul(
                        out=psum_ek[:, b * K:(b + 1) * K],
                        lhsT=lhs, rhs=keysT16[:, b * K:(b + 1) * K],
                        start=True, stop=True)
                ekb = ekpool.tile([8, N], f32, tag="ekb")
                nc.vector.tensor_add(ekb[:, :], psum_ek[:, :],
                                     mb_sb[:, gidx * N:(gidx + 1) * N])
                flat_view = bass.AP(
                    tensor=ekflat.tensor, offset=ekflat.offset + s0 * N,
                    ap=[ekflat.ap[0], [N, 8], [1, N]])
                nc.sync.dma_start(out=flat_view, in_=ekb[:, :])
                ek_groups_done += 1

        j0 = 0
        for nb in CHUNKS:
            for j in range(j0, j0 + nb):
                nc.gpsimd.indirect_dma_start(
                    out=G_sb[0:TPB, j * D:(j + 1) * D],
                    out_offset=None,
                    in_=d_emb[:, :],
                    in_offset=bass.IndirectOffsetOnAxis(ap=idx_sb[:, j:j + 1], axis=0),
                )
                w = min(6, S * BL - 6 * j)
                wt = wpool.tile([TPB, D], f16, tag="wt")
                nc.gpsimd.tensor_tensor(
                    out=wt[:, :], in0=G_sb[0:TPB, j * D:(j + 1) * D],
                    in1=posw, op=OP.mult)
                nc.tensor.matmul(
                    out=psum_enc[:, 6 * j:6 * j + w],
                    lhsT=wt[:, :], rhs=omap16[:, 0:w],
                    start=True, stop=True)
            c0, c1 = 6 * j0, min(6 * (j0 + nb), S * BL)
            nc.scalar.copy(out=enc16[:, c0:c1], in_=psum_enc[:, c0:c1])
            # E_rep for these columns: broadcast each enc column over k
            src = bass.AP(tensor=enc16.tensor, offset=enc16.offset + c0,
                          ap=[enc16.ap[0], [1, c1 - c0], [0, K]])
            dst = bass.AP(tensor=E_rep.tensor, offset=E_rep.offset + c0 * K,
                          ap=[E_rep.ap[0], [K, c1 - c0], [1, K]])
            nc.vector.tensor_copy(out=dst, in_=src)
            cols_done = c1
            emit_ek_groups(cols_done)
            j0 += nb

        # ---- Phase B: 64 recurrence steps ----
        for s in range(S):
            # gate: q = ones16^T @ (h16 * E_s) + ek_row_s
            P16 = step_sb.tile([D, N], f16, tag="P16")
            nc.vector.tensor_mul(P16[:, :], h16[:, :],
                                 E_rep[:, s * N:(s + 1) * N])
            psum_q = p_q.tile([D, N], f32, tag="q")
            nc.tensor.matmul(out=psum_q[:, :], lhsT=row1,
                             rhs=ekflat[0:1, s * N:(s + 1) * N],
                             start=True, stop=False)
            nc.tensor.matmul(out=psum_q[:, :], lhsT=ones16, rhs=P16[:, :],
                             start=False, stop=True)
            g = step_sb.tile([D, N], f16, tag="g")
            nc.scalar.activation(g[:, :], psum_q[:, :], AF.Sigmoid)

            # candidate: HT = relu(U^T h16 + V^T keysT + W^T e_rep_s)
            psum_ht = p_ht.tile([D, N], f32, tag="ht")
            nc.tensor.matmul(out=psum_ht[:, :], lhsT=U16, rhs=h16[:, :],
                             start=True, stop=False)
            nc.tensor.matmul(out=psum_ht[:, :], lhsT=V16, rhs=keysT16,
                             start=False, stop=False)
            nc.tensor.matmul(out=psum_ht[:, :], lhsT=W16,
                             rhs=E_rep[:, s * N:(s + 1) * N],
                             start=False, stop=True)
            HT = step_sb.tile([D, N], f16, tag="HT")
            nc.scalar.activation(HT[:, :], psum_ht[:, :], AF.Relu)

            # update: y = h + g*HT (all fp16: DVE 2x mode)
            Tg = step_sb.tile([D, N], f16, tag="Tg")
            nc.vector.tensor_mul(Tg[:, :], g[:, :], HT[:, :])
            y = step_sb.tile([D, N], f16, tag="y")
            nc.vector.tensor_add(y[:, :], h16[:, :], Tg[:, :])
            SQ16 = step_sb.tile([D, N], f16, tag="SQ16")
            nc.vector.tensor_mul(SQ16[:, :], y[:, :], y[:, :])

            # norm: hs = 0.5*(ss + eps) accumulated directly in PSUM
            psum_ss = p_ss.tile([D, N], f32, tag="ss")
            nc.tensor.matmul(out=psum_ss[:, :], lhsT=row1, rhs=epsrow,
                             start=True, stop=False)
            nc.tensor.matmul(out=psum_ss[:, :], lhsT=half16, rhs=SQ16[:, :],
                             start=False, stop=True)
            sh = step_sb.tile([D, N], i32, tag="sh")
            nc.vector.tensor_scalar(out=sh[:, :], in0=psum_ss[:, :].bitcast(i32),
                                    scalar1=1, scalar2=None,
                                    op0=OP.logical_shift_right)
            r0 = step_sb.tile([D, N], f32, tag="r0")
            nc.vector.tensor_scalar(out=r0[:, :].bitcast(i32), in0=sh[:, :],
                                    scalar1=-1, scalar2=M2,
                                    op0=OP.mult, op1=OP.add)
            t0 = step_sb.tile([D, N], f32, tag="t0")
            nc.vector.tensor_mul(t0[:, :], psum_ss[:, :], r0[:, :])
            b0 = step_sb.tile([D, N], f32, tag="b0")
            nc.vector.tensor_mul(b0[:, :], t0[:, :], r0[:, :])
            q1 = step_sb.tile([D, N], f32, tag="q1")   # = -r1
            nc.vector.scalar_tensor_tensor(out=q1[:, :], in0=b0[:, :],
                                           scalar=1.5, in1=r0[:, :],
                                           op0=OP.subtract, op1=OP.mult)
            if NR_ITERS == 2:
                t1 = step_sb.tile([D, N], f32, tag="t1")
                nc.vector.tensor_mul(t1[:, :], psum_ss[:, :], q1[:, :])
                b1 = step_sb.tile([D, N], f32, tag="b1")
                nc.vector.tensor_mul(b1[:, :], t1[:, :], q1[:, :])
                rsf = step_sb.tile([D, N], f32, tag="q2")  # = +r2
                nc.vector.scalar_tensor_tensor(out=rsf[:, :], in0=b1[:, :],
                                               scalar=1.5, in1=q1[:, :],
                                               op0=OP.subtract, op1=OP.mult)
                rs16 = step_sb.tile([D, N], f16, tag="rs16")
                # clamp: rs can reach 1.4e6 (eps-floored zero columns) which
                # would overflow fp16 -> Inf -> 0*Inf = NaN. rs>100 only ever
                # pairs with y~0, so the clamp is mathematically inert.
                nc.vector.tensor_scalar(out=rs16[:, :], in0=rsf[:, :],
                                        scalar1=60000.0, scalar2=None,
                                        op0=OP.min)
            else:
                rs16 = step_sb.tile([D, N], f16, tag="rs16")  # = -r1 in fp16
                nc.vector.tensor_scalar(out=rs16[:, :], in0=q1[:, :],
                                        scalar1=-60000.0, scalar2=None,
                                        op0=OP.max)
            h16_new = hpool.tile([D, N], f16, tag="h16")
            if NR_ITERS == 2:
                nc.vector.tensor_mul(h16_new[:, :], y[:, :], rs16[:, :])
            else:
                nc.vector.scalar_tensor_tensor(out=h16_new[:, :], in0=y[:, :],
                                               scalar=-1.0, in1=rs16[:, :],
                                               op0=OP.mult, op1=OP.mult)
            h16 = h16_new

        hout = singles.tile([D, N], f32)
        nc.vector.tensor_copy(out=hout[:, :], in_=h16[:, :])
        nc.sync.dma_start(out=d_out[:, :], in_=hout[:, :])

    return nc


def _legalize_waits(bir_json: bytes) -> bytes:
    """Walrus codegen allows at most ONE sem-wait per instruction; Tile's sem
    assigner emits several. Hoist all but the last wait onto EventSemaphore
    carrier instructions inserted just before the offender."""
    import orjson
    bir = orjson.loads(bir_json)
    n_new = 0
    for fn in bir.get("functions", []):
        for bb in fn.get("blocks", []):
            out = []
            for inst in bb.get("instructions", []):
                si = inst.get("sync_info") or {}
                ow = si.get("on_wait") or []
                if len(ow) > 1:
                    for w in ow[:-1]:
                        n_new += 1
                        out.append({
                            "debug": inst.get("debug", 0),
                            "engine": inst["engine"],
                            "ins": [], "outs": [],
                            "name": f"waitfix_{n_new}_{inst.get('name','')}",
                            "opcode": "EventSemaphore",
                            "sync_info": {"on_update": [], "on_wait": [w]},
                        })
                    si["on_wait"] = [ow[-1]]
                    inst["sync_info"] = si
                out.append(inst)
            bb["instructions"] = out
    return orjson.dumps(bir)


def _install_compile_hook():
    import concourse.bass2jax as b2j
    if getattr(b2j, "_waitfix_installed", False):
        return
    orig = b2j.compile_bir_kernel

    def patched(bir_json, tmpdir, neff_name="file.neff"):
        return orig(_legalize_waits(bir_json), tmpdir, neff_name)

    b2j.compile_bir_kernel = patched
    b2j._waitfix_installed = True


def get_nc():
    if "nc" not in _NC_CACHE:
        _NC_CACHE["nc"] = _build_nc()
    return _NC_CACHE["nc"]


def make_shared_consts(embedding_matrix, pos_mask):
    f: type = np.float32
    emb_aug = np.vstack([np.asarray(embedding_matrix, dtype=f),
                         np.zeros((1, D), dtype=f)])
    emb_aug = np.ascontiguousarray(emb_aug)

    pm = np.asarray(pos_mask, dtype=f)
    c32 = np.zeros((128, C32W), dtype=f)
    for p in range(TPB):
        c32[p, C32_POSW:C32_POSW + 128] = pm[p % L]
    c32[0, C32_ROW:C32_ROW + 128] = 1.0
    c32[0, C32_EPS:C32_EPS + N] = 0.5e-12
    return emb_aug, c32


def make_inputs_for_core(c, prgrph, prgrph_mask, keys, emb_aug, U, V, W, c32):
    f = np.float32
    bsl = slice(c * BL, (c + 1) * BL)
    pr = np.asarray(prgrph[bsl]).astype(np.int64)      # [BL, S, L]
    mk = np.asarray(prgrph_mask[bsl]).astype(bool)     # [BL, S, L]
    ky = np.asarray(keys[bsl], dtype=f)                # [BL, K, D]

    # token order: t = ((s*BL + b)*L + l); block j = t//TPB, row t%TPB
    idx_flat = np.where(mk, pr, ZROW).transpose(1, 0, 2).reshape(-1).astype(np.int32)
    idx_pad = np.full(NBLK * TPB, ZROW, dtype=np.int32)
    idx_pad[:TOK] = idx_flat
    idx2d = np.ascontiguousarray(idx_pad.reshape(NBLK, TPB).T)  # [TPB, NBLK]

    keysT = ky.transpose(2, 0, 1).reshape(D, N)

    c16 = np.zeros((128, C16W), dtype=np.float16)
    c16[:, C16_ONES:C16_ONES + 128] = 1.0
    c16[:, C16_HALF:C16_HALF + 128] = 0.5
    c16[:, C16_U:C16_U + 128] = np.asarray(U, dtype=f)
    c16[:, C16_V:C16_V + 128] = np.asarray(V, dtype=f)
    c16[:, C16_W:C16_W + 128] = np.asarray(W, dtype=f)
    c16[:, C16_KEYS:C16_KEYS + N] = keysT
    for p in range(TPB):
        c16[p, C16_OMAP + p // L] = 1.0

    m_s = mk[:, :, 0].astype(f)                         # [BL, S]
    mb = (m_s.T - 1.0) * (-GATE_BIAS)                   # [S, BL]: 0 or GATE_BIAS
    mb = np.repeat(mb[:, :, None], K, axis=2).reshape(S, N).astype(f)
    # pre-swizzle so EK group g reads rows [0:8] at partition base 0:
    # mb_g[r, g*N+n] = mb[g*8+r, n]
    mb = mb.reshape(S // 8, 8, N).transpose(1, 0, 2).reshape(8, (S // 8) * N)

    return {
        "emb": emb_aug,
        "idx": idx2d,
        "mb": np.ascontiguousarray(mb),
        "c16": np.ascontiguousarray(c16),
        "c32": np.ascontiguousarray(c32),
    }


def kernel(prgrph, prgrph_mask, keys, embedding_matrix, pos_mask, U, V, W):
    from concourse.bass_utils import run_bass_kernel_spmd
    _install_compile_hook()

    emb_aug, c32 = make_shared_consts(embedding_matrix, pos_mask)
    in_maps = [
        make_inputs_for_core(c, prgrph, prgrph_mask, keys, emb_aug, U, V, W, c32)
        for c in range(NCORES)
    ]
    nc = get_nc()
    res = run_bass_kernel_spmd(nc, in_maps, core_ids=list(range(NCORES)))
    outs = []
    for c in range(NCORES):
        o = np.asarray(res.results[c]["out"])        # [D, N]
        outs.append(o.T.reshape(BL, K, D))
    return np.concatenate(outs, axis=0).astype(np.float32)
